# revision 29
# baseline (speedup 1.0000x reference)
"""CuboidSelfAttention Trainium2 kernel (v2 — pipelined).

Problem: x (2, 8, 112, 112, 256) fp32 -> LayerNorm -> cuboid reorder
(2x7x7 cuboids => 2048 independent cuboids of 98 tokens) -> 8-head self
attention within each cuboid -> out projection -> inverse reorder.

Sharding: 2048 cuboids split across 8 cores (256 each); weights replicated.

v2 changes vs v1 (which ran 2.11 ms):
  - x staged to device in bf16 (halves input DMA, LN stats fine in bf16).
  - LN normalize moved from GPSIMD (4 us/op) to ACT:
    out = Identity(x * rr + (-mean*rr)) with per-partition scale/bias.
  - scores PSUM shrunk [128,4,2,256]->[128,4,2,128] (4 banks -> 2) and
    double-buffered so PE can run cuboid i+1's scores while ACT does
    exp(i): PE/ACT ping-pong was the main serializer.
  - LN of macro-chunk m+1 issued interleaved with attention groups of
    chunk m (separate PSUM tags; no shared-ring false deps).
  - recip reads sums straight from PSUM; recip broadcast is bf16 and its
    DMAs are dispatched on the gpsimd queue (v1 put them on the scalar
    queue ahead of exp -> ACT starved); y-out DMA on scalar right after
    its own ACT evac; x-in split across sync/gpsimd queues.
  - no zero-bias adds (plain copies when has_beta false), kt unpadded
    (scores matmuls use M=98), y output in bf16.

Per-core dataflow (feature-major), 4 macro-chunks of 64 cuboids:
  LN: token-major bf16 tiles (bn_stats/bn_aggr, batched rsqrt),
      ACT-normalize to bf16, PE-transpose -> xnT [256, 6272] bf16.
  attention, per group of 16 cuboids, sub-chunks of 4:
    A: qT/kT = Wq/Wk^T @ xnT, scores^T per head via row-packed K=32
       matmuls, exp on ACT, softmax sums via ones-column matmuls
       accumulated into one PSUM tile (row j = cuboid j's sums).
    recip of 16 cuboids' sums in one DVE op (PSUM src) -> DRAM scratch.
    B: v token-major, AV col-packed (oT feature-major), normalize-evac
       with broadcast recip (DMA), proj, ACT evac, DMA out.
  host: transpose + inverse cuboid reorder.
"""

import numpy as np
import ml_dtypes

import concourse.bass as bass
import concourse.bacc as bacc
import concourse.mybir as mybir
import concourse.tile as tile
from concourse.bass_utils import run_bass_kernel_spmd

# ---------------- problem constants (hardcoded) ----------------
B, T, H, W, C = 2, 8, 112, 112, 256
HEADS = 8
DH = C // HEADS  # 32
CT, CH, CW = 2, 7, 7
CV = CT * CH * CW  # 98
NT, NH, NW = T // CT, H // CH, W // CW  # 4, 16, 16
NOC = NT * NH * NW  # 1024
NCUB = B * NOC  # 2048
NCORES = 8
CUB_PC = NCUB // NCORES  # 256 cuboids per core
TOK_PC = CUB_PC * CV  # 25088 tokens per core
EPS = 1e-6

# device tiling
MC_CUB = 64              # cuboids per macro chunk
N_MC = CUB_PC // MC_CUB  # 4
MC_TOK = MC_CUB * CV     # 6272
N_TT = MC_TOK // 128     # 49 token tiles per macro chunk
G16 = 16                 # cuboids per sums group
SUB = 4                  # cuboids per sub-chunk
SUB_TOK = SUB * CV       # 392

F32 = mybir.dt.float32
BF16 = mybir.dt.bfloat16

# LN tile slices interleaved with the 4 attention groups: 49 = 13+12+12+12
LN_SLICES = [(0, 13), (13, 25), (25, 37), (37, 49)]

_prog_cache = {}


def _build_program(has_beta: bool, has_pbias: bool):
    nc = bacc.Bacc("TRN2")

    x_d = nc.dram_tensor("x", [TOK_PC, C], BF16, kind="ExternalInput")
    wq_d = nc.dram_tensor("wq", [128, 2, C], BF16, kind="ExternalInput")
    wk_d = nc.dram_tensor("wk", [128, 2, C], BF16, kind="ExternalInput")
    wv_d = nc.dram_tensor("wv", [128, 2, C], BF16, kind="ExternalInput")
    wp_d = nc.dram_tensor("wp", [128, 2, C], BF16, kind="ExternalInput")
    qkbias_d = nc.dram_tensor("qkbias", [128, 4], F32, kind="ExternalInput")
    pbias_d = nc.dram_tensor("pbias", [128, 2], F32, kind="ExternalInput")
    vbias_d = nc.dram_tensor("vbias", [1, C], BF16, kind="ExternalInput")
    onescol_d = nc.dram_tensor("onescol", [CV, 32 * 128], BF16, kind="ExternalInput")
    ident_d = nc.dram_tensor("ident", [128, 128], BF16, kind="ExternalInput")
    ones98_d = nc.dram_tensor("ones98", [1, CV], BF16, kind="ExternalInput")
    y_d = nc.dram_tensor("y", [C, TOK_PC], BF16, kind="ExternalOutput")

    from contextlib import ExitStack

    with tile.TileContext(nc) as tc:
        with ExitStack() as ctx:
            ep = ctx.enter_context
            consts = ep(tc.tile_pool(name="consts", bufs=1))
            xin_p = ep(tc.tile_pool(name="xin", bufs=20))
            stats_p = ep(tc.tile_pool(name="stats", bufs=2))
            xn_p = ep(tc.tile_pool(name="xn", bufs=6))
            xnt_p = ep(tc.tile_pool(name="xnt", bufs=3))
            qt_p = ep(tc.tile_pool(name="qt", bufs=2))
            kt_p = ep(tc.tile_pool(name="kt", bufs=2))
            attn_p = ep(tc.tile_pool(name="attn", bufs=20))
            rsb_p = ep(tc.tile_pool(name="rsb", bufs=2))
            recip_d_p = ep(tc.tile_pool(name="recip_dram", bufs=2, space="DRAM"))
            rb_p = ep(tc.tile_pool(name="rb", bufs=5))
            v_p = ep(tc.tile_pool(name="vsb", bufs=3))
            on_p = ep(tc.tile_pool(name="on", bufs=10))
            out_p = ep(tc.tile_pool(name="osb", bufs=4))
            # PSUM: sc 2 bufs x 2 banks + sums 1 bank + gp 3 x 1 bank = 8
            sc_ps = ep(tc.tile_pool(name="sc_ps", bufs=2, space="PSUM"))
            sums_ps_p = ep(tc.tile_pool(name="sums_ps", bufs=1, space="PSUM"))
            gp_ps = ep(tc.tile_pool(name="gp_ps", bufs=3, space="PSUM"))

            # ---- constants ----
            wq_sb = consts.tile([128, 2, C], BF16)
            wk_sb = consts.tile([128, 2, C], BF16)
            wv_sb = consts.tile([128, 2, C], BF16)
            wp_sb = consts.tile([128, 2, C], BF16)
            nc.sync.dma_start(out=wq_sb, in_=wq_d[:, :, :])
            nc.sync.dma_start(out=wk_sb, in_=wk_d[:, :, :])
            nc.sync.dma_start(out=wv_sb, in_=wv_d[:, :, :])
            nc.sync.dma_start(out=wp_sb, in_=wp_d[:, :, :])
            qkb_sb = consts.tile([128, 4], F32)
            nc.sync.dma_start(out=qkb_sb, in_=qkbias_d[:, :])
            pb_sb = consts.tile([128, 2], F32)
            nc.sync.dma_start(out=pb_sb, in_=pbias_d[:, :])
            onescol_sb = consts.tile([CV, 32 * 128], BF16)
            nc.sync.dma_start(out=onescol_sb, in_=onescol_d[:, :])
            ident_sb = consts.tile([128, 128], BF16)
            nc.sync.dma_start(out=ident_sb, in_=ident_d[:, :])
            vb_sb = consts.tile([1, C], BF16)
            nc.sync.dma_start(out=vb_sb, in_=vbias_d[:, :])
            ones98_sb = consts.tile([1, CV], BF16)
            nc.sync.dma_start(out=ones98_sb, in_=ones98_d[:, :])
            eps_sb = consts.tile([128, 1], F32)
            nc.vector.memset(eps_sb, EPS)

            xnts = [None] * N_MC

            def ln_slice(mc, sl):
                """LN + transpose for token tiles [t0, t1) of macro chunk mc."""
                t0s, t1s = LN_SLICES[sl]
                if sl == 0:
                    xnts[mc] = xnt_p.tile([128, 2, MC_TOK], BF16, name="xnt")
                xnt = xnts[mc]
                mc_tok0 = mc * MC_TOK
                n = t1s - t0s
                mv = stats_p.tile([128, n, 2], F32, tag="mv", padded_shape=[128, 13, 2])
                rr = stats_p.tile([128, n], F32, tag="rr", padded_shape=[128, 13])
                xts = []
                for i in range(n):
                    tt = t0s + i
                    xt = xin_p.tile([128, C], BF16)
                    eng = nc.sync if (tt % 2 == 0) else nc.gpsimd
                    eng.dma_start(
                        out=xt,
                        in_=x_d[mc_tok0 + tt * 128 : mc_tok0 + (tt + 1) * 128, :],
                    )
                    st = stats_p.tile([128, 6], F32, tag="st", bufs=3)
                    nc.vector.bn_stats(out=st, in_=xt)
                    nc.vector.bn_aggr(out=mv[:, i, :], in_=st)
                    xts.append(xt)
                # rr = 1/sqrt(var + eps); nmr = -mean * rr
                nc.scalar.activation(
                    out=rr,
                    in_=mv[:, :, 1],
                    func=mybir.ActivationFunctionType.Sqrt,
                    bias=eps_sb,
                    scale=1.0,
                )
                nc.vector.reciprocal(out=rr, in_=rr)
                for i in range(n):
                    tt = t0s + i
                    xn = xn_p.tile([128, C], BF16)
                    nc.vector.tensor_scalar(
                        out=xn,
                        in0=xts[i],
                        scalar1=mv[:, i, 0:1],
                        scalar2=rr[:, i : i + 1],
                        op0=mybir.AluOpType.subtract,
                        op1=mybir.AluOpType.mult,
                    )
                    tp = gp_ps.tile([128, 2, 128], BF16, tag="gp")
                    nc.tensor.transpose(tp[:, 0, :], xn[:, 0:128], ident_sb)
                    nc.tensor.transpose(tp[:, 1, :], xn[:, 128:256], ident_sb)
                    nc.vector.tensor_copy(
                        out=xnt[:, :, tt * 128 : (tt + 1) * 128], in_=tp
                    )

            def attn_group(mc, g):
                """Attention for group g (16 cuboids) of macro chunk mc."""
                xnt = xnts[mc]
                mc_tok0 = mc * MC_TOK
                sums_ps = sums_ps_p.tile([128, SUB_TOK], F32, name="sums_ps")
                attns = []
                for s in range(4):  # sub-chunks of 4 cuboids
                    t0 = g * G16 * CV + s * SUB_TOK
                    # --- q/k projections, feature-major ---
                    qt = qt_p.tile([128, 2, SUB_TOK], BF16)
                    kt = kt_p.tile([128, 2, SUB, CV], BF16)
                    for which, dst_q in ((0, True), (1, False)):
                        w_sb = wq_sb if dst_q else wk_sb
                        for mt in range(2):
                            ps = gp_ps.tile([128, SUB_TOK], F32, tag="gp")
                            for ktile in range(2):
                                nc.tensor.matmul(
                                    ps,
                                    lhsT=w_sb[:, ktile, mt * 128 : (mt + 1) * 128],
                                    rhs=xnt[:, ktile, t0 : t0 + SUB_TOK],
                                    start=(ktile == 0),
                                    stop=(ktile == 1),
                                )
                            dst = (
                                qt[:, mt, :]
                                if dst_q
                                else kt[:, mt, :, :].rearrange("p c v -> p (c v)")
                            )
                            # q evac on DVE, k evac on ACT (engine balance)
                            if has_beta:
                                bias = qkb_sb[:, which * 2 + mt : which * 2 + mt + 1]
                                if dst_q:
                                    nc.vector.tensor_scalar(
                                        out=dst,
                                        in0=ps,
                                        scalar1=bias,
                                        scalar2=None,
                                        op0=mybir.AluOpType.add,
                                    )
                                else:
                                    nc.scalar.activation(
                                        out=dst,
                                        in_=ps,
                                        func=mybir.ActivationFunctionType.Identity,
                                        bias=bias,
                                        scale=1.0,
                                    )
                            elif dst_q:
                                nc.vector.tensor_copy(out=dst, in_=ps)
                            else:
                                nc.scalar.copy(out=dst, in_=ps)
                    # --- scores + exp + sums, per cuboid ---
                    # attn memory head order h' = rg*2 + grp (orig h = grp*4+rg)
                    # so each rg-pair's exp covers a contiguous 4-head block:
                    # exp(pair p) runs on banks 2p..2p+1 while PE fills the
                    # other bank pair of the next cuboid (scps bufs=1).
                    for ci in range(4):
                        c16 = s * 4 + ci  # cuboid index within group of 16
                        # One 2-bank tile per rg pair (bank = rg_local):
                        # concurrent row-group MMs hit different PSUM banks;
                        # same-rg MMs serialize on the PE row group, so
                        # sharing a bank across grp is safe. bufs=2 ring ->
                        # pair 0 of cuboid i+1 only waits on exp(i, pair 0).
                        attn = attn_p.tile([128, HEADS, CV], BF16)
                        for pair in range(2):
                            scps = sc_ps.tile([128, 2, 2, 256], F32, name="scps")
                            for rgl in range(2):
                                rg = 2 * pair + rgl
                                for grp in range(2):
                                    nc.tensor.matmul(
                                        scps[0:CV, rgl, grp, 0:CV],
                                        lhsT=kt[rg * 32 : (rg + 1) * 32, grp, ci, :],
                                        rhs=qt[
                                            rg * 32 : (rg + 1) * 32,
                                            grp,
                                            ci * CV : (ci + 1) * CV,
                                        ],
                                        tile_position=(rg * 32, 0),
                                    )
                            nc.scalar.activation(
                                out=attn[
                                    0:CV, 4 * pair : 4 * pair + 4, :
                                ].rearrange("p (h g) v -> p h g v", g=2),
                                in_=scps[0:CV, :, :, 0:CV],
                                func=mybir.ActivationFunctionType.Exp,
                            )
                        attns.append(attn)
                        # sums: row (p*16 + c16) of sums_ps accumulates the
                        # 4-head block p (memory order)
                        for p in range(2):
                            j = p * 16 + c16
                            nc.tensor.matmul(
                                sums_ps,
                                lhsT=onescol_sb[:, j * 128 : (j + 1) * 128],
                                rhs=attn[0:CV, p * 4 : (p + 1) * 4, :].rearrange(
                                    "p h v -> p (h v)"
                                ),
                                start=(c16 == 0 and p == 0),
                                stop=(c16 == 15 and p == 1),
                                skip_group_check=True,
                            )
                # --- reciprocal of all 16 cuboids' sums -> DRAM scratch ---
                rsb = rsb_p.tile([32, SUB_TOK], F32)
                nc.vector.reciprocal_approx_fast(out=rsb, in_=sums_ps[0:32, :])
                recip_dram = recip_d_p.tile([32, SUB_TOK], F32)
                nc.sync.dma_start(out=recip_dram, in_=rsb)

                # --- B1: v + AV for all sub-chunks first, so PE has ~5us
                # of matmuls to chew while the recip->DRAM->rb broadcast
                # chain completes; proj (which needs the normalized o) runs
                # in B2 after. Without the split the first proj MM
                # head-blocks the PE queue ~8.6us every group (HAM goes
                # cold each time).
                ons_all = []
                for s in range(4):
                    t0 = g * G16 * CV + s * SUB_TOK
                    # broadcast recips: rb[p=(hh,d), c, grp, q], fp32.
                    # head h = grp*4+hh lives at memory slot h' = hh*2+grp
                    # -> recip row j = (hh//2)*16 + c16, col ((hh%2)*2+grp)*CV.
                    # (grp, q) is a contiguous 196-col block per hh, so one
                    # 3-dim DMA per hh covers (d, ci, grp*q).
                    rb = rb_p.tile([128, SUB, 2, CV], F32)
                    for hh in range(4):
                        src = recip_dram[0, :]
                        src_b = bass.AP(
                            tensor=src.tensor,
                            offset=src.offset
                            + ((hh // 2) * 16 + s * 4) * SUB_TOK
                            + (hh % 2) * 2 * CV,
                            ap=[
                                [0, 32],        # d replicate
                                [SUB_TOK, 4],   # ci -> row
                                [1, 2 * CV],    # (grp, q) contiguous
                            ],
                        )
                        nc.gpsimd.dma_start(
                            out=rb[hh * 32 : (hh + 1) * 32, :, :, :],
                            in_=src_b,
                        )
                    # v projection, token-major (2 cuboids per PSUM bank)
                    vsb = v_p.tile([CV, SUB, C], BF16)
                    for vh in range(2):
                        vps = gp_ps.tile([CV, 2, C], F32, tag="gp")
                        for cj in range(2):
                            ci = vh * 2 + cj
                            for ktile in range(2):
                                nc.tensor.matmul(
                                    vps[:, cj, :],
                                    lhsT=xnt[
                                        :, ktile, t0 + ci * CV : t0 + (ci + 1) * CV
                                    ],
                                    rhs=wv_sb[:, ktile, :],
                                    start=(ktile == 0),
                                    stop=(ktile == 1) and not has_beta,
                                )
                            if has_beta:
                                nc.tensor.matmul(
                                    vps[:, cj, :],
                                    lhsT=ones98_sb,
                                    rhs=vb_sb,
                                    start=False,
                                    stop=True,
                                )
                        nc.vector.tensor_copy(
                            out=vsb[:, vh * 2 : (vh + 1) * 2, :], in_=vps
                        )
                    # AV: col-packed, oT feature-major + normalize evac
                    ons = []
                    for grp in range(2):
                        otps = gp_ps.tile([128, SUB, CV], F32, tag="gp")
                        for ci in range(4):
                            for cg in range(4):
                                nc.tensor.matmul(
                                    otps[cg * 32 : (cg + 1) * 32, ci, :],
                                    lhsT=vsb[
                                        :,
                                        ci,
                                        grp * 128 + cg * 32 : grp * 128 + (cg + 1) * 32,
                                    ],
                                    # head grp*4+cg is at memory slot cg*2+grp
                                    rhs=attns[s * 4 + ci][0:CV, cg * 2 + grp, :],
                                    tile_position=(0, cg * 32),
                                )
                        on = on_p.tile([128, SUB, CV], BF16)
                        nc.vector.tensor_tensor(
                            out=on,
                            in0=otps,
                            in1=rb[:, :, grp, :],
                            op=mybir.AluOpType.mult,
                        )
                        ons.append(on)
                    ons_all.append(ons)

                # --- B2: proj + evac + out-DMA ---
                for s in range(4):
                    t0 = g * G16 * CV + s * SUB_TOK
                    ons = ons_all[s]
                    for mt in range(2):
                        pps = gp_ps.tile([128, SUB_TOK], F32, tag="gp")
                        for ktile in range(2):
                            nc.tensor.matmul(
                                pps,
                                lhsT=wp_sb[:, ktile, mt * 128 : (mt + 1) * 128],
                                rhs=ons[ktile].rearrange("p c v -> p (c v)"),
                                start=(ktile == 0),
                                stop=(ktile == 1),
                            )
                        osb = out_p.tile([128, SUB_TOK], BF16)
                        if has_pbias:
                            nc.scalar.activation(
                                out=osb,
                                in_=pps,
                                func=mybir.ActivationFunctionType.Identity,
                                bias=pb_sb[:, mt : mt + 1],
                                scale=1.0,
                            )
                        else:
                            nc.scalar.copy(out=osb, in_=pps)
                        nc.scalar.dma_start(
                            out=y_d[
                                mt * 128 : (mt + 1) * 128,
                                mc_tok0 + t0 : mc_tok0 + t0 + SUB_TOK,
                            ],
                            in_=osb,
                        )

            # ---- main schedule: LN(mc+1) interleaved with attention(mc) ----
            for sl in range(4):
                ln_slice(0, sl)
            for mc in range(N_MC):
                for g in range(4):
                    if mc + 1 < N_MC:
                        ln_slice(mc + 1, g)
                    attn_group(mc, g)
    nc.finalize()
    return nc


# ---------------- host-side helpers ----------------

def _cuboid_fwd(x):
    """(B, T, H, W, C) -> (NCUB, CV, C)"""
    xr = x.reshape(B, NT, CT, NH, CH, NW, CW, C)
    xr = xr.transpose(0, 1, 3, 5, 2, 4, 6, 7)
    return np.ascontiguousarray(xr.reshape(NCUB, CV, C))


def _cuboid_inv(o):
    """(NCUB, CV, C) -> (B, T, H, W, C)"""
    o = o.reshape(B, NT, NH, NW, CT, CH, CW, C)
    o = o.transpose(0, 1, 4, 2, 5, 3, 6, 7)
    return np.ascontiguousarray(o.reshape(B, T, H, W, C))


def _prep_consts(ln_scale, ln_bias, w_qkv, w_proj, b_proj):
    bf = ml_dtypes.bfloat16
    scale = np.float32(DH) ** np.float32(-0.5)
    wg = (w_qkv.astype(np.float32) * ln_scale.astype(np.float32)[:, None])
    wq = wg[:, 0:C] * scale
    wk = wg[:, C : 2 * C]
    wv = wg[:, 2 * C : 3 * C]
    qkv_bias = ln_bias.astype(np.float32) @ w_qkv.astype(np.float32)
    qb = qkv_bias[0:C] * scale
    kb = qkv_bias[C : 2 * C]
    vb = qkv_bias[2 * C : 3 * C]
    has_beta = bool(np.any(vb != 0.0) or np.any(qb != 0.0) or np.any(kb != 0.0))
    has_pbias = bool(np.any(b_proj != 0.0))

    def ktiles(w):  # (256, 256) -> (128, 2, 256)
        return np.ascontiguousarray(
            w.reshape(2, 128, C).transpose(1, 0, 2)
        ).astype(bf)

    consts = {
        "wq": ktiles(wq),
        "wk": ktiles(wk),
        "wv": ktiles(wv),
        "wp": ktiles(w_proj.astype(np.float32)),
        "qkbias": np.ascontiguousarray(
            np.stack(
                [qb[0:128], qb[128:256], kb[0:128], kb[128:256]], axis=1
            )
        ).astype(np.float32),
        "pbias": np.ascontiguousarray(
            b_proj.astype(np.float32).reshape(2, 128).T
        ),
        "vbias": vb.reshape(1, C).astype(bf),
        "ones98": np.ones((1, CV), dtype=bf),
        "ident": np.eye(128, dtype=np.float32).astype(bf),
    }
    onescol = np.zeros((CV, 32, 128), dtype=np.float32)
    for j in range(32):
        onescol[:, j, j] = 1.0
    consts["onescol"] = onescol.reshape(CV, 32 * 128).astype(bf)
    return consts, has_beta, has_pbias


def _run(inputs, trace=False, tmpdir=None):
    bf = ml_dtypes.bfloat16
    x = np.asarray(inputs["x"], dtype=np.float32)
    consts, has_beta, has_pbias = _prep_consts(
        np.asarray(inputs["ln_scale"], np.float32),
        np.asarray(inputs["ln_bias"], np.float32),
        np.asarray(inputs["w_qkv"], np.float32),
        np.asarray(inputs["w_proj"], np.float32),
        np.asarray(inputs["b_proj"], np.float32),
    )
    key = (has_beta, has_pbias)
    if key not in _prog_cache:
        _prog_cache[key] = _build_program(has_beta, has_pbias)
    nc = _prog_cache[key]

    xc = _cuboid_fwd(x)  # (2048, 98, 256)
    in_maps = []
    for core in range(NCORES):
        xcore = np.ascontiguousarray(
            xc[core * CUB_PC : (core + 1) * CUB_PC].reshape(TOK_PC, C)
        ).astype(bf)
        m = {"x": xcore}
        m.update(consts)
        in_maps.append(m)

    res = run_bass_kernel_spmd(
        nc,
        in_maps,
        core_ids=list(range(NCORES)),
        trace=trace,
        tmpdir=tmpdir,
    )
    outs = []
    for core in range(NCORES):
        y = res.results[core]["y"]  # (256, 25088) bf16 feature-major
        outs.append(np.asarray(y, dtype=np.float32).T.reshape(CUB_PC, CV, C))
    o = np.concatenate(outs, axis=0)
    return _cuboid_inv(o).astype(np.float32), res


def kernel(**inputs) -> np.ndarray:
    out, _ = _run(inputs, trace=False)
    return out


# revision 35
# speedup vs baseline: 1.2087x; 1.2087x over previous
"""CuboidSelfAttention Trainium2 kernel (v2 — pipelined).

Problem: x (2, 8, 112, 112, 256) fp32 -> LayerNorm -> cuboid reorder
(2x7x7 cuboids => 2048 independent cuboids of 98 tokens) -> 8-head self
attention within each cuboid -> out projection -> inverse reorder.

Sharding: 2048 cuboids split across 8 cores (256 each); weights replicated.

v2 changes vs v1 (which ran 2.11 ms):
  - x staged to device in bf16 (halves input DMA, LN stats fine in bf16).
  - LN normalize moved from GPSIMD (4 us/op) to ACT:
    out = Identity(x * rr + (-mean*rr)) with per-partition scale/bias.
  - scores PSUM shrunk [128,4,2,256]->[128,4,2,128] (4 banks -> 2) and
    double-buffered so PE can run cuboid i+1's scores while ACT does
    exp(i): PE/ACT ping-pong was the main serializer.
  - LN of macro-chunk m+1 issued interleaved with attention groups of
    chunk m (separate PSUM tags; no shared-ring false deps).
  - recip reads sums straight from PSUM; recip broadcast is bf16 and its
    DMAs are dispatched on the gpsimd queue (v1 put them on the scalar
    queue ahead of exp -> ACT starved); y-out DMA on scalar right after
    its own ACT evac; x-in split across sync/gpsimd queues.
  - no zero-bias adds (plain copies when has_beta false), kt unpadded
    (scores matmuls use M=98), y output in bf16.

Per-core dataflow (feature-major), 4 macro-chunks of 64 cuboids:
  LN: token-major bf16 tiles (bn_stats/bn_aggr, batched rsqrt),
      ACT-normalize to bf16, PE-transpose -> xnT [256, 6272] bf16.
  attention, per group of 16 cuboids, sub-chunks of 4:
    A: qT/kT = Wq/Wk^T @ xnT, scores^T per head via row-packed K=32
       matmuls, exp on ACT, softmax sums via ones-column matmuls
       accumulated into one PSUM tile (row j = cuboid j's sums).
    recip of 16 cuboids' sums in one DVE op (PSUM src) -> DRAM scratch.
    B: v token-major, AV col-packed (oT feature-major), normalize-evac
       with broadcast recip (DMA), proj, ACT evac, DMA out.
  host: transpose + inverse cuboid reorder.
"""

import numpy as np
import ml_dtypes

import concourse.bass as bass
import concourse.bacc as bacc
import concourse.mybir as mybir
import concourse.tile as tile
from concourse.bass_utils import run_bass_kernel_spmd

# ---------------- problem constants (hardcoded) ----------------
B, T, H, W, C = 2, 8, 112, 112, 256
HEADS = 8
DH = C // HEADS  # 32
CT, CH, CW = 2, 7, 7
CV = CT * CH * CW  # 98
NT, NH, NW = T // CT, H // CH, W // CW  # 4, 16, 16
NOC = NT * NH * NW  # 1024
NCUB = B * NOC  # 2048
NCORES = 8
CUB_PC = NCUB // NCORES  # 256 cuboids per core
TOK_PC = CUB_PC * CV  # 25088 tokens per core
EPS = 1e-6

# device tiling
MC_CUB = 64              # cuboids per macro chunk
N_MC = CUB_PC // MC_CUB  # 4
MC_TOK = MC_CUB * CV     # 6272
N_TT = MC_TOK // 128     # 49 token tiles per macro chunk
G16 = 16                 # cuboids per sums group
SUB = 4                  # cuboids per sub-chunk
SUB_TOK = SUB * CV       # 392

F32 = mybir.dt.float32
BF16 = mybir.dt.bfloat16

# LN tile slices interleaved with the 4 attention groups: 49 = 13+12+12+12
LN_SLICES = [(0, 13), (13, 25), (25, 37), (37, 49)]

_prog_cache = {}


def _build_program(has_beta: bool, has_pbias: bool):
    nc = bacc.Bacc("TRN2")

    x_d = nc.dram_tensor("x", [TOK_PC, C], BF16, kind="ExternalInput")
    wq_d = nc.dram_tensor("wq", [128, 2, C], BF16, kind="ExternalInput")
    wk_d = nc.dram_tensor("wk", [128, 2, C], BF16, kind="ExternalInput")
    wv_d = nc.dram_tensor("wv", [128, 2, C], BF16, kind="ExternalInput")
    wp_d = nc.dram_tensor("wp", [128, 2, C], BF16, kind="ExternalInput")
    qkbias_d = nc.dram_tensor("qkbias", [128, 4], F32, kind="ExternalInput")
    pbias_d = nc.dram_tensor("pbias", [128, 2], F32, kind="ExternalInput")
    vbias_d = nc.dram_tensor("vbias", [1, C], BF16, kind="ExternalInput")
    onescol_d = nc.dram_tensor("onescol", [CV, 32 * 128], BF16, kind="ExternalInput")
    ident_d = nc.dram_tensor("ident", [128, 128], BF16, kind="ExternalInput")
    ones98_d = nc.dram_tensor("ones98", [1, CV], BF16, kind="ExternalInput")
    y_d = nc.dram_tensor("y", [C, TOK_PC], BF16, kind="ExternalOutput")

    from contextlib import ExitStack

    with tile.TileContext(nc) as tc:
        with ExitStack() as ctx:
            ep = ctx.enter_context
            consts = ep(tc.tile_pool(name="consts", bufs=1))
            xin_p = ep(tc.tile_pool(name="xin", bufs=20))
            stats_p = ep(tc.tile_pool(name="stats", bufs=2))
            xn_p = ep(tc.tile_pool(name="xn", bufs=6))
            xnt_p = ep(tc.tile_pool(name="xnt", bufs=3))
            qt_p = ep(tc.tile_pool(name="qt", bufs=2))
            kt_p = ep(tc.tile_pool(name="kt", bufs=2))
            attn_p = ep(tc.tile_pool(name="attn", bufs=20))
            rsb_p = ep(tc.tile_pool(name="rsb", bufs=4))
            recip_d_p = ep(tc.tile_pool(name="recip_dram", bufs=6, space="DRAM"))
            rb_p = ep(tc.tile_pool(name="rb", bufs=6))
            v_p = ep(tc.tile_pool(name="vsb", bufs=3))
            on_p = ep(tc.tile_pool(name="on", bufs=4))
            out_p = ep(tc.tile_pool(name="osb", bufs=4))
            # PSUM: sc 2 bufs x 2 banks + sums 2 x 1 bank + gp 2 x 1 bank = 8
            sc_ps = ep(tc.tile_pool(name="sc_ps", bufs=2, space="PSUM"))
            sums_ps_p = ep(tc.tile_pool(name="sums_ps", bufs=2, space="PSUM"))
            gp_ps = ep(tc.tile_pool(name="gp_ps", bufs=2, space="PSUM"))

            # ---- constants ----
            wq_sb = consts.tile([128, 2, C], BF16)
            wk_sb = consts.tile([128, 2, C], BF16)
            wv_sb = consts.tile([128, 2, C], BF16)
            wp_sb = consts.tile([128, 2, C], BF16)
            nc.sync.dma_start(out=wq_sb, in_=wq_d[:, :, :])
            nc.sync.dma_start(out=wk_sb, in_=wk_d[:, :, :])
            nc.sync.dma_start(out=wv_sb, in_=wv_d[:, :, :])
            nc.sync.dma_start(out=wp_sb, in_=wp_d[:, :, :])
            qkb_sb = consts.tile([128, 4], F32)
            nc.sync.dma_start(out=qkb_sb, in_=qkbias_d[:, :])
            pb_sb = consts.tile([128, 2], F32)
            nc.sync.dma_start(out=pb_sb, in_=pbias_d[:, :])
            onescol_sb = consts.tile([CV, 32 * 128], BF16)
            nc.sync.dma_start(out=onescol_sb, in_=onescol_d[:, :])
            ident_sb = consts.tile([128, 128], BF16)
            nc.sync.dma_start(out=ident_sb, in_=ident_d[:, :])
            vb_sb = consts.tile([1, C], BF16)
            nc.sync.dma_start(out=vb_sb, in_=vbias_d[:, :])
            ones98_sb = consts.tile([1, CV], BF16)
            nc.sync.dma_start(out=ones98_sb, in_=ones98_d[:, :])
            eps_sb = consts.tile([128, 1], F32)
            nc.vector.memset(eps_sb, EPS)

            xnts = [None] * N_MC

            def ln_slice(mc, sl):
                """LN + transpose for token tiles [t0, t1) of macro chunk mc."""
                t0s, t1s = LN_SLICES[sl]
                if sl == 0:
                    xnts[mc] = xnt_p.tile([128, 2, MC_TOK], BF16, name="xnt")
                xnt = xnts[mc]
                mc_tok0 = mc * MC_TOK
                n = t1s - t0s
                mv = stats_p.tile([128, n, 2], F32, tag="mv", padded_shape=[128, 13, 2])
                rr = stats_p.tile([128, n], F32, tag="rr", padded_shape=[128, 13])
                xts = []
                for i in range(n):
                    tt = t0s + i
                    xt = xin_p.tile([128, C], BF16)
                    nc.sync.dma_start(
                        out=xt,
                        in_=x_d[mc_tok0 + tt * 128 : mc_tok0 + (tt + 1) * 128, :],
                    )
                    st = stats_p.tile([128, 6], F32, tag="st", bufs=3)
                    nc.vector.bn_stats(out=st, in_=xt)
                    nc.vector.bn_aggr(out=mv[:, i, :], in_=st)
                    xts.append(xt)
                # rr = 1/sqrt(var + eps); nmr = -mean * rr
                nc.scalar.activation(
                    out=rr,
                    in_=mv[:, :, 1],
                    func=mybir.ActivationFunctionType.Sqrt,
                    bias=eps_sb,
                    scale=1.0,
                )
                nc.vector.reciprocal(out=rr, in_=rr)
                for i in range(n):
                    tt = t0s + i
                    xn = xn_p.tile([128, C], BF16)
                    nc.vector.tensor_scalar(
                        out=xn,
                        in0=xts[i],
                        scalar1=mv[:, i, 0:1],
                        scalar2=rr[:, i : i + 1],
                        op0=mybir.AluOpType.subtract,
                        op1=mybir.AluOpType.mult,
                    )
                    tp = gp_ps.tile([128, 2, 128], BF16, tag="gp")
                    nc.tensor.transpose(tp[:, 0, :], xn[:, 0:128], ident_sb)
                    nc.tensor.transpose(tp[:, 1, :], xn[:, 128:256], ident_sb)
                    nc.vector.tensor_copy(
                        out=xnt[:, :, tt * 128 : (tt + 1) * 128], in_=tp
                    )

            def attn_group(mc, g):
                """Attention for group g (16 cuboids) of macro chunk mc."""
                xnt = xnts[mc]
                mc_tok0 = mc * MC_TOK
                attns = []
                rbs = []
                for s in range(4):  # sub-chunks of 4 cuboids
                    t0 = g * G16 * CV + s * SUB_TOK
                    # per-sub-chunk softmax sums (8 rows: p*4+ci) so the
                    # recip -> DRAM -> broadcast chain for sub-chunk s runs
                    # during A(s+1) and rb is ready before the B phase.
                    sums_ps = sums_ps_p.tile(
                        [128, SUB_TOK], F32, name="sums_ps"
                    )
                    # --- q/k projections, feature-major ---
                    qt = qt_p.tile([128, 2, SUB_TOK], BF16)
                    kt = kt_p.tile([128, 2, SUB, CV], BF16)
                    for which, dst_q in ((0, True), (1, False)):
                        w_sb = wq_sb if dst_q else wk_sb
                        for mt in range(2):
                            ps = gp_ps.tile([128, SUB_TOK], F32, tag="gp")
                            for ktile in range(2):
                                nc.tensor.matmul(
                                    ps,
                                    lhsT=w_sb[:, ktile, mt * 128 : (mt + 1) * 128],
                                    rhs=xnt[:, ktile, t0 : t0 + SUB_TOK],
                                    start=(ktile == 0),
                                    stop=(ktile == 1),
                                )
                            dst = (
                                qt[:, mt, :]
                                if dst_q
                                else kt[:, mt, :, :].rearrange("p c v -> p (c v)")
                            )
                            # q evac on DVE, k evac on ACT (engine balance)
                            if has_beta:
                                bias = qkb_sb[:, which * 2 + mt : which * 2 + mt + 1]
                                if dst_q:
                                    nc.vector.tensor_scalar(
                                        out=dst,
                                        in0=ps,
                                        scalar1=bias,
                                        scalar2=None,
                                        op0=mybir.AluOpType.add,
                                    )
                                else:
                                    nc.scalar.activation(
                                        out=dst,
                                        in_=ps,
                                        func=mybir.ActivationFunctionType.Identity,
                                        bias=bias,
                                        scale=1.0,
                                    )
                            elif dst_q:
                                nc.vector.tensor_copy(out=dst, in_=ps)
                            else:
                                nc.scalar.copy(out=dst, in_=ps)
                    # --- scores + exp + sums, per cuboid ---
                    # attn memory head order h' = rg*2 + grp (orig h = grp*4+rg)
                    # so each rg-pair's exp covers a contiguous 4-head block:
                    # exp(pair p) runs on banks 2p..2p+1 while PE fills the
                    # other bank pair of the next cuboid (scps bufs=1).
                    for ci in range(4):
                        c16 = s * 4 + ci  # cuboid index within group of 16
                        # One 2-bank tile per rg pair (bank = rg_local):
                        # concurrent row-group MMs hit different PSUM banks;
                        # same-rg MMs serialize on the PE row group, so
                        # sharing a bank across grp is safe. bufs=2 ring ->
                        # pair 0 of cuboid i+1 only waits on exp(i, pair 0).
                        attn = attn_p.tile([128, HEADS, CV], BF16)
                        for pair in range(2):
                            scps = sc_ps.tile([128, 2, 2, 256], F32, name="scps")
                            for rgl in range(2):
                                rg = 2 * pair + rgl
                                for grp in range(2):
                                    nc.tensor.matmul(
                                        scps[0:CV, rgl, grp, 0:CV],
                                        lhsT=kt[rg * 32 : (rg + 1) * 32, grp, ci, :],
                                        rhs=qt[
                                            rg * 32 : (rg + 1) * 32,
                                            grp,
                                            ci * CV : (ci + 1) * CV,
                                        ],
                                        tile_position=(rg * 32, 0),
                                    )
                            nc.scalar.activation(
                                out=attn[
                                    0:CV, 4 * pair : 4 * pair + 4, :
                                ].rearrange("p (h g) v -> p h g v", g=2),
                                in_=scps[0:CV, :, :, 0:CV],
                                func=mybir.ActivationFunctionType.Exp,
                            )
                        attns.append(attn)
                        # sums: row (p*4 + ci) of sums_ps accumulates the
                        # 4-head block p (memory order)
                        for p in range(2):
                            j = p * 4 + ci
                            nc.tensor.matmul(
                                sums_ps,
                                lhsT=onescol_sb[:, j * 128 : (j + 1) * 128],
                                rhs=attn[0:CV, p * 4 : (p + 1) * 4, :].rearrange(
                                    "p h v -> p (h v)"
                                ),
                                start=(ci == 0 and p == 0),
                                stop=(ci == 3 and p == 1),
                                skip_group_check=True,
                            )
                    # recip of this sub-chunk's sums -> DRAM -> broadcast.
                    rsb = rsb_p.tile([8, SUB_TOK], F32)
                    nc.vector.reciprocal_approx_fast(
                        out=rsb, in_=sums_ps[0:8, :]
                    )
                    recip_dram = recip_d_p.tile([8, SUB_TOK], F32)
                    nc.sync.dma_start(out=recip_dram, in_=rsb)
                    # rb[p=(hh,d), c, grp, q], fp32. head h = grp*4+hh is at
                    # memory slot h' = hh*2+grp -> row j = (hh//2)*4 + ci,
                    # col ((hh%2)*2+grp)*CV; (grp, q) contiguous 196 cols.
                    rb = rb_p.tile([128, SUB, 2, CV], F32)
                    for hh in range(4):
                        src = recip_dram[0, :]
                        src_b = bass.AP(
                            tensor=src.tensor,
                            offset=src.offset
                            + (hh // 2) * 4 * SUB_TOK
                            + (hh % 2) * 2 * CV,
                            ap=[
                                [0, 32],        # d replicate
                                [SUB_TOK, 4],   # ci -> row
                                [1, 2 * CV],    # (grp, q) contiguous
                            ],
                        )
                        nc.gpsimd.dma_start(
                            out=rb[hh * 32 : (hh + 1) * 32, :, :, :],
                            in_=src_b,
                        )
                    rbs.append(rb)

                # --- B: v, AV, normalize, proj, out ---
                for s in range(4):
                    t0 = g * G16 * CV + s * SUB_TOK
                    rb = rbs[s]
                    # v projection, token-major (2 cuboids per PSUM bank)
                    vsb = v_p.tile([CV, SUB, C], BF16)
                    for vh in range(2):
                        vps = gp_ps.tile([CV, 2, C], F32, tag="gp")
                        for cj in range(2):
                            ci = vh * 2 + cj
                            for ktile in range(2):
                                nc.tensor.matmul(
                                    vps[:, cj, :],
                                    lhsT=xnt[
                                        :, ktile, t0 + ci * CV : t0 + (ci + 1) * CV
                                    ],
                                    rhs=wv_sb[:, ktile, :],
                                    start=(ktile == 0),
                                    stop=(ktile == 1) and not has_beta,
                                )
                            if has_beta:
                                nc.tensor.matmul(
                                    vps[:, cj, :],
                                    lhsT=ones98_sb,
                                    rhs=vb_sb,
                                    start=False,
                                    stop=True,
                                )
                        nc.vector.tensor_copy(
                            out=vsb[:, vh * 2 : (vh + 1) * 2, :], in_=vps
                        )
                    # AV: col-packed, oT feature-major + normalize evac
                    ons = []
                    for grp in range(2):
                        otps = gp_ps.tile([128, SUB, CV], F32, tag="gp")
                        for ci in range(4):
                            for cg in range(4):
                                nc.tensor.matmul(
                                    otps[cg * 32 : (cg + 1) * 32, ci, :],
                                    lhsT=vsb[
                                        :,
                                        ci,
                                        grp * 128 + cg * 32 : grp * 128 + (cg + 1) * 32,
                                    ],
                                    # head grp*4+cg is at memory slot cg*2+grp
                                    rhs=attns[s * 4 + ci][0:CV, cg * 2 + grp, :],
                                    tile_position=(0, cg * 32),
                                )
                        on = on_p.tile([128, SUB, CV], BF16)
                        nc.vector.tensor_tensor(
                            out=on,
                            in0=otps,
                            in1=rb[:, :, grp, :],
                            op=mybir.AluOpType.mult,
                        )
                        ons.append(on)
                    # proj
                    for mt in range(2):
                        pps = gp_ps.tile([128, SUB_TOK], F32, tag="gp")
                        for ktile in range(2):
                            nc.tensor.matmul(
                                pps,
                                lhsT=wp_sb[:, ktile, mt * 128 : (mt + 1) * 128],
                                rhs=ons[ktile].rearrange("p c v -> p (c v)"),
                                start=(ktile == 0),
                                stop=(ktile == 1),
                            )
                        osb = out_p.tile([128, SUB_TOK], BF16)
                        if has_pbias:
                            nc.scalar.activation(
                                out=osb,
                                in_=pps,
                                func=mybir.ActivationFunctionType.Identity,
                                bias=pb_sb[:, mt : mt + 1],
                                scale=1.0,
                            )
                        else:
                            nc.scalar.copy(out=osb, in_=pps)
                        nc.scalar.dma_start(
                            out=y_d[
                                mt * 128 : (mt + 1) * 128,
                                mc_tok0 + t0 : mc_tok0 + t0 + SUB_TOK,
                            ],
                            in_=osb,
                        )

            # ---- main schedule: LN(mc+1) interleaved with attention(mc) ----
            for sl in range(4):
                ln_slice(0, sl)
            for mc in range(N_MC):
                for g in range(4):
                    if mc + 1 < N_MC:
                        ln_slice(mc + 1, g)
                    attn_group(mc, g)
    nc.finalize()
    return nc


# ---------------- host-side helpers ----------------

def _cuboid_fwd(x):
    """(B, T, H, W, C) -> (NCUB, CV, C)"""
    xr = x.reshape(B, NT, CT, NH, CH, NW, CW, C)
    xr = xr.transpose(0, 1, 3, 5, 2, 4, 6, 7)
    return np.ascontiguousarray(xr.reshape(NCUB, CV, C))


def _cuboid_inv(o):
    """(NCUB, CV, C) -> (B, T, H, W, C)"""
    o = o.reshape(B, NT, NH, NW, CT, CH, CW, C)
    o = o.transpose(0, 1, 4, 2, 5, 3, 6, 7)
    return np.ascontiguousarray(o.reshape(B, T, H, W, C))


def _prep_consts(ln_scale, ln_bias, w_qkv, w_proj, b_proj):
    bf = ml_dtypes.bfloat16
    scale = np.float32(DH) ** np.float32(-0.5)
    wg = (w_qkv.astype(np.float32) * ln_scale.astype(np.float32)[:, None])
    wq = wg[:, 0:C] * scale
    wk = wg[:, C : 2 * C]
    wv = wg[:, 2 * C : 3 * C]
    qkv_bias = ln_bias.astype(np.float32) @ w_qkv.astype(np.float32)
    qb = qkv_bias[0:C] * scale
    kb = qkv_bias[C : 2 * C]
    vb = qkv_bias[2 * C : 3 * C]
    has_beta = bool(np.any(vb != 0.0) or np.any(qb != 0.0) or np.any(kb != 0.0))
    has_pbias = bool(np.any(b_proj != 0.0))

    def ktiles(w):  # (256, 256) -> (128, 2, 256)
        return np.ascontiguousarray(
            w.reshape(2, 128, C).transpose(1, 0, 2)
        ).astype(bf)

    consts = {
        "wq": ktiles(wq),
        "wk": ktiles(wk),
        "wv": ktiles(wv),
        "wp": ktiles(w_proj.astype(np.float32)),
        "qkbias": np.ascontiguousarray(
            np.stack(
                [qb[0:128], qb[128:256], kb[0:128], kb[128:256]], axis=1
            )
        ).astype(np.float32),
        "pbias": np.ascontiguousarray(
            b_proj.astype(np.float32).reshape(2, 128).T
        ),
        "vbias": vb.reshape(1, C).astype(bf),
        "ones98": np.ones((1, CV), dtype=bf),
        "ident": np.eye(128, dtype=np.float32).astype(bf),
    }
    onescol = np.zeros((CV, 32, 128), dtype=np.float32)
    for j in range(32):
        onescol[:, j, j] = 1.0
    consts["onescol"] = onescol.reshape(CV, 32 * 128).astype(bf)
    return consts, has_beta, has_pbias


def _run(inputs, trace=False, tmpdir=None):
    bf = ml_dtypes.bfloat16
    x = np.asarray(inputs["x"], dtype=np.float32)
    consts, has_beta, has_pbias = _prep_consts(
        np.asarray(inputs["ln_scale"], np.float32),
        np.asarray(inputs["ln_bias"], np.float32),
        np.asarray(inputs["w_qkv"], np.float32),
        np.asarray(inputs["w_proj"], np.float32),
        np.asarray(inputs["b_proj"], np.float32),
    )
    key = (has_beta, has_pbias)
    if key not in _prog_cache:
        _prog_cache[key] = _build_program(has_beta, has_pbias)
    nc = _prog_cache[key]

    xc = _cuboid_fwd(x)  # (2048, 98, 256)
    in_maps = []
    for core in range(NCORES):
        xcore = np.ascontiguousarray(
            xc[core * CUB_PC : (core + 1) * CUB_PC].reshape(TOK_PC, C)
        ).astype(bf)
        m = {"x": xcore}
        m.update(consts)
        in_maps.append(m)

    res = run_bass_kernel_spmd(
        nc,
        in_maps,
        core_ids=list(range(NCORES)),
        trace=trace,
        tmpdir=tmpdir,
    )
    outs = []
    for core in range(NCORES):
        y = res.results[core]["y"]  # (256, 25088) bf16 feature-major
        outs.append(np.asarray(y, dtype=np.float32).T.reshape(CUB_PC, CV, C))
    o = np.concatenate(outs, axis=0)
    return _cuboid_inv(o).astype(np.float32), res


def kernel(**inputs) -> np.ndarray:
    out, _ = _run(inputs, trace=False)
    return out
